# revision 13
# baseline (speedup 1.0000x reference)
"""Trainium2 Bass kernel for masked average pooling (AvgPoolingInitializer).

Computes, per example b:
    fg_init[b,i,c] = mean of fmap[b,c,:,:] over pixels where fg_mask[b,i,:,:] > 0.5
    bg_init[b,q,c] = mean of fmap[b,c,:,:] over pixels where bg_mask[b,q,:,:] > 0.5
                     (zeros if the mask is empty)

Sharding: pure data parallel, one example per NeuronCore (B=8 examples, 8 cores).

Per-core algorithm (n = H*W = 16384 flattened pixels):
  - masks [20, n] are PE-transposed in chunks of 128, then thresholded in fp32
    straight out of PSUM (>0.5 -> 1.0/0.0) into an fp16 wT slab [n-chunk, 20].
  - fmap [256, n] is streamed in natural layout, each [128c, 128n] tile is
    PE-transposed (fp32, exact) into PSUM, copied+rounded to fp16 in SBUF
    (alternating DVE/ACT), and contracted against wT on the PE:
        acc[20, 256] += wT_chunk.T @ fT_chunk   (fp16 in, fp32 accumulate)
        cnt[20, 1]   += wT_chunk.T @ ones       (fp32, exact)
  - epilogue: out = acc * 1/max(cnt, 1), DMA to fg_out/bg_out.

`reps` repeats the whole per-example pipeline inside one NEFF; used by the
timing harness to measure per-iteration device time as a slope, cancelling
the constant dispatch/tunnel overhead.
"""

import numpy as np

B = 8
C = 256
H = W = 128
N = H * W          # 16384
I_FG = 16
Q_BG = 4
K20 = I_FG + Q_BG  # 20 combined mask rows
CH = 128           # contraction chunk width (PE partition dim)
NCHUNK = N // CH   # 128 chunks
SC = 8             # superchunks (DMA granularity: 1 MiB per c-block)
NSC = N // SC      # 2048 columns per superchunk
CPS = NSC // CH    # 16 chunks per superchunk
MB = 8             # mask chunks batched per PSUM tile

_nc_cache = {}
_runner_cache = {}


def _build_nc(reps=1):
    import concourse.bacc as bacc
    import concourse.tile as tile
    import concourse.mybir as mybir
    from concourse.masks import make_identity
    from contextlib import ExitStack

    F16 = mybir.dt.float16
    F32 = mybir.dt.float32

    nc = bacc.Bacc("TRN2")
    fmap = nc.dram_tensor("fmap", [C, N], F32, kind="ExternalInput")
    fg = nc.dram_tensor("fg", [I_FG, N], F32, kind="ExternalInput")
    bg = nc.dram_tensor("bg", [Q_BG, N], F32, kind="ExternalInput")
    fg_out = nc.dram_tensor("fg_out", [I_FG, C], F32, kind="ExternalOutput")
    bg_out = nc.dram_tensor("bg_out", [Q_BG, C], F32, kind="ExternalOutput")

    with tile.TileContext(nc) as tc, ExitStack() as ctx:
        singles = ctx.enter_context(tc.tile_pool(name="singles", bufs=1))
        per_rep = ctx.enter_context(tc.tile_pool(name="per_rep", bufs=min(2, reps)))
        fpool = ctx.enter_context(tc.tile_pool(name="fpool", bufs=2))
        rhspool = ctx.enter_context(tc.tile_pool(name="rhs", bufs=4))
        psA = ctx.enter_context(tc.tile_pool(name="psA", bufs=min(2, reps), space="PSUM"))
        psT = ctx.enter_context(tc.tile_pool(name="psT", bufs=2, space="PSUM"))
        psM = ctx.enter_context(tc.tile_pool(name="psM", bufs=2, space="PSUM"))
        psW = ctx.enter_context(tc.tile_pool(name="psW", bufs=1, space="PSUM"))

        ident = singles.tile([128, 128], F32)
        make_identity(nc, ident)
        ones = singles.tile([128, 1], F16)
        nc.vector.memset(ones, 1.0)

        # dummy transpose so the PE's vector clock covers the identity's
        # producer (gpsimd) before the mask DMAs land — keeps every matmul's
        # LDWEIGHTS at <=1 sync wait after bacc's event-semaphore split.
        warm = psW.tile([128, MB * K20], F32)
        nc.tensor.transpose(warm[:, 0:128], ident[:, :], ident[:, :])

        mask_nat = singles.tile([K20, N], F32)

        for rep in range(reps):
            # ---- masks: load natural, transpose on PE, threshold from PSUM ----
            nc.sync.dma_start(out=mask_nat[0:I_FG, :], in_=fg[:, :])
            nc.sync.dma_start(out=mask_nat[I_FG:K20, :], in_=bg[:, :])

            wT = per_rep.tile([128, NCHUNK * K20], F16)

            acc = psA.tile([K20, C], F32)
            cnt = psW.tile([K20, 1], F32)

            for batch in range(NCHUNK // MB):
                mt = psM.tile([128, MB * K20], F32)
                for i in range(MB):
                    g = batch * MB + i
                    nc.tensor.transpose(
                        mt[:, i * K20 : (i + 1) * K20],
                        mask_nat[0:K20, g * CH : (g + 1) * CH],
                        ident[0:K20, 0:K20],
                    )
                lo, hi = batch * MB * K20, (batch + 1) * MB * K20
                # threshold in fp32 straight out of PSUM; 0/1 is exact in fp16
                nc.vector.tensor_scalar(
                    out=wT[:, lo:hi],
                    in0=mt[:, :],
                    scalar1=0.5,
                    scalar2=None,
                    op0=mybir.AluOpType.is_gt,
                )

            # ---- main loop: stream f, transpose on PE, matmul-accumulate ----
            for sc in range(SC):
                ft = fpool.tile([128, 2, NSC], F32)
                for cb in range(2):
                    nc.sync.dma_start(
                        out=ft[:, cb, :],
                        in_=fmap[cb * 128 : (cb + 1) * 128, sc * NSC : (sc + 1) * NSC],
                    )
                for jp in range(CPS // 2):  # chunks in pairs
                    pt = psT.tile([128, 2, C], F32)  # one full PSUM bank
                    for u in range(2):
                        j = jp * 2 + u
                        for cb in range(2):
                            nc.tensor.transpose(
                                pt[:, u, cb * 128 : (cb + 1) * 128],
                                ft[:, cb, j * CH : (j + 1) * CH],
                                ident[:, :],
                            )
                    rhs = rhspool.tile([128, 2, C], F16)
                    if jp % 2 == 0:
                        nc.vector.tensor_copy(rhs[:, :, :], pt[:, :, :])
                    else:
                        nc.scalar.copy(rhs[:, :, :], pt[:, :, :])
                    for u in range(2):
                        g = sc * CPS + jp * 2 + u
                        first, last = g == 0, g == NCHUNK - 1
                        nc.tensor.matmul(
                            acc[:, :],
                            lhsT=wT[:, g * K20 : (g + 1) * K20],
                            rhs=rhs[:, u, :],
                            start=first,
                            stop=last,
                        )
                        nc.tensor.matmul(
                            cnt[:, :],
                            lhsT=wT[:, g * K20 : (g + 1) * K20],
                            rhs=ones[:, :],
                            start=first,
                            stop=last,
                        )

            # ---- epilogue: divide by max(cnt, 1) ----
            res = per_rep.tile([K20, C], F32)
            csb = per_rep.tile([K20, 1], F32)
            rec = per_rep.tile([K20, 1], F32)
            nc.vector.tensor_copy(res[:, :], acc[:, :])
            nc.vector.tensor_copy(csb[:, :], cnt[:, :])
            nc.vector.tensor_scalar_max(csb, csb, 1.0)
            nc.vector.reciprocal(rec, csb)
            nc.vector.tensor_scalar_mul(res, res, rec[:, :])
            nc.sync.dma_start(out=fg_out[:, :], in_=res[0:I_FG, :])
            nc.sync.dma_start(out=bg_out[:, :], in_=res[I_FG:K20, :])

    nc.compile()
    return nc


def _get_nc(reps=1):
    if reps not in _nc_cache:
        _nc_cache[reps] = _build_nc(reps)
    return _nc_cache[reps]


def _make_runner(reps=1):
    """Build a cached callable (fmap, fg, bg full np arrays) -> jax outputs.
    Keeps device-resident inputs and one compiled executable so repeated calls
    measure device time + small dispatch overhead only."""
    if reps in _runner_cache:
        return _runner_cache[reps]

    import jax
    from jax.sharding import Mesh, PartitionSpec
    from jax.experimental.shard_map import shard_map
    from concourse import bass2jax

    bass2jax.install_neuronx_cc_hook()
    nc = _get_nc(reps)

    in_names = ["fmap", "fg", "bg"]
    out_names = ["fg_out", "bg_out"]
    out_avals = [
        jax.core.ShapedArray((I_FG, C), np.float32),
        jax.core.ShapedArray((Q_BG, C), np.float32),
    ]
    all_in_names = in_names + out_names
    if nc.partition_id_tensor is not None:
        all_in_names.append(nc.partition_id_tensor.name)

    def _body(*args):
        operands = list(args)
        if nc.partition_id_tensor is not None:
            operands.append(bass2jax.partition_id_tensor())
        outs = bass2jax._bass_exec_p.bind(
            *operands,
            out_avals=tuple(out_avals),
            in_names=tuple(all_in_names),
            out_names=tuple(out_names),
            lowering_input_output_aliases=(),
            sim_require_finite=True,
            sim_require_nnan=True,
            nc=nc,
        )
        return tuple(outs)

    devices = jax.devices()[:B]
    mesh = Mesh(np.asarray(devices), ("core",))
    n_in = len(in_names) + len(out_names)
    donate = tuple(range(len(in_names), n_in))
    sharded = jax.jit(
        shard_map(
            _body,
            mesh=mesh,
            in_specs=(PartitionSpec("core"),) * n_in,
            out_specs=(PartitionSpec("core"),) * len(out_names),
            check_rep=False,
        ),
        donate_argnums=donate,
        keep_unused=True,
    )

    state = {}

    def runner(fmap, fg_mask, bg_mask):
        key = (fmap.ctypes.data, fg_mask.ctypes.data, bg_mask.ctypes.data)
        if state.get("key") != key:
            state["key"] = key
            state["in"] = [
                jax.device_put(fmap.reshape(B * C, N)),
                jax.device_put(fg_mask.reshape(B * I_FG, N)),
                jax.device_put(bg_mask.reshape(B * Q_BG, N)),
            ]
        zeros = [
            np.zeros((B * I_FG, C), np.float32),
            np.zeros((B * Q_BG, C), np.float32),
        ]
        outs = sharded(*state["in"], *zeros)
        jax.block_until_ready(outs)
        return outs

    _runner_cache[reps] = runner
    return runner


def run(fmap, fg_mask, bg_mask, **spmd_kwargs):
    """Run on 8 NeuronCores via run_bass_kernel_spmd; returns
    (fg_init, bg_init, BassKernelResults)."""
    from concourse.bass_utils import run_bass_kernel_spmd

    fmap = np.ascontiguousarray(np.asarray(fmap, dtype=np.float32))
    fg_mask = np.ascontiguousarray(np.asarray(fg_mask, dtype=np.float32))
    bg_mask = np.ascontiguousarray(np.asarray(bg_mask, dtype=np.float32))
    assert fmap.shape == (B, C, H, W)
    assert fg_mask.shape == (B, I_FG, H, W)
    assert bg_mask.shape == (B, Q_BG, H, W)

    in_maps = [
        {
            "fmap": fmap[b].reshape(C, N),
            "fg": fg_mask[b].reshape(I_FG, N),
            "bg": bg_mask[b].reshape(Q_BG, N),
        }
        for b in range(B)
    ]
    nc = _get_nc(1)
    res = run_bass_kernel_spmd(nc, in_maps, core_ids=list(range(B)), **spmd_kwargs)
    fg_init = np.stack([r["fg_out"] for r in res.results]).astype(np.float32)
    bg_init = np.stack([r["bg_out"] for r in res.results]).astype(np.float32)
    return fg_init, bg_init, res


def kernel(fmap, fg_mask, bg_mask):
    fg_init, bg_init, _ = run(fmap, fg_mask, bg_mask)
    return fg_init, bg_init


# revision 21
# speedup vs baseline: 7.9732x; 7.9732x over previous
"""Trainium2 Bass kernel for masked average pooling (AvgPoolingInitializer).

Computes, per example b:
    fg_init[b,i,c] = mean of fmap[b,c,:,:] over pixels where fg_mask[b,i,:,:] > 0.5
    bg_init[b,q,c] = mean of fmap[b,c,:,:] over pixels where bg_mask[b,q,:,:] > 0.5
                     (zeros if the mask is empty)

Sharding: pure data parallel, one example per NeuronCore (B=8 examples, 8 cores).

Per-core algorithm (n = H*W = 16384 flattened pixels):
  - masks [20, n] are PE-transposed in chunks of 128, then thresholded in fp32
    straight out of PSUM (>0.5 -> 1.0/0.0) into an fp16 wT slab [n-chunk, 20].
  - fmap [256, n] is streamed in natural layout, each [128c, 128n] tile is
    PE-transposed (fp32, exact) into PSUM, copied+rounded to fp16 in SBUF
    (alternating DVE/ACT), and contracted against wT on the PE:
        acc[20, 256] += wT_chunk.T @ fT_chunk   (fp16 in, fp32 accumulate)
        cnt[20, 1]   += wT_chunk.T @ ones       (fp32, exact)
  - epilogue: out = acc * 1/max(cnt, 1), DMA to fg_out/bg_out.

`reps` repeats the whole per-example pipeline inside one NEFF; used by the
timing harness to measure per-iteration device time as a slope, cancelling
the constant dispatch/tunnel overhead.
"""

import numpy as np

B = 8
C = 256
H = W = 128
N = H * W          # 16384
I_FG = 16
Q_BG = 4
K20 = I_FG + Q_BG  # 20 combined mask rows
CH = 128           # contraction chunk width (PE partition dim)
NCHUNK = N // CH   # 128 chunks
SC = 4             # superchunks (DMA granularity: 2 MiB per c-block)
NSC = N // SC      # 2048 columns per superchunk
CPS = NSC // CH    # 16 chunks per superchunk
MB = 8             # mask chunks batched per PSUM tile

_nc_cache = {}
_runner_cache = {}


def _build_nc(reps=1):
    import concourse.bacc as bacc
    import concourse.tile as tile
    import concourse.mybir as mybir
    from concourse.masks import make_identity
    from contextlib import ExitStack

    F16 = mybir.dt.float16
    F32 = mybir.dt.float32

    nc = bacc.Bacc("TRN2")
    fmap = nc.dram_tensor("fmap", [C, N], F32, kind="ExternalInput")
    fg = nc.dram_tensor("fg", [I_FG, N], F32, kind="ExternalInput")
    bg = nc.dram_tensor("bg", [Q_BG, N], F32, kind="ExternalInput")
    fg_out = nc.dram_tensor("fg_out", [I_FG, C], F32, kind="ExternalOutput")
    bg_out = nc.dram_tensor("bg_out", [Q_BG, C], F32, kind="ExternalOutput")

    with tile.TileContext(nc) as tc, ExitStack() as ctx:
        singles = ctx.enter_context(tc.tile_pool(name="singles", bufs=1))
        per_rep = ctx.enter_context(tc.tile_pool(name="per_rep", bufs=min(2, reps)))
        fpool = ctx.enter_context(tc.tile_pool(name="fpool", bufs=3))
        rhspool = ctx.enter_context(tc.tile_pool(name="rhs", bufs=4))
        psA = ctx.enter_context(tc.tile_pool(name="psA", bufs=1, space="PSUM"))
        psT = ctx.enter_context(tc.tile_pool(name="psT", bufs=3, space="PSUM"))
        psM = ctx.enter_context(tc.tile_pool(name="psM", bufs=2, space="PSUM"))
        psW = ctx.enter_context(tc.tile_pool(name="psW", bufs=1, space="PSUM"))

        ident = singles.tile([128, 128], F32)
        make_identity(nc, ident)
        ident16 = singles.tile([128, 128], F16)
        nc.vector.tensor_copy(ident16[:, :], ident[:, :])
        ones = singles.tile([128, 1], F16)
        nc.vector.memset(ones, 1.0)

        # dummy transpose so the PE's vector clock covers the identity's
        # producer (gpsimd) before the mask DMAs land — keeps every matmul's
        # LDWEIGHTS at <=1 sync wait after bacc's event-semaphore split.
        warm = psW.tile([128, MB * K20], F32)
        nc.tensor.transpose(warm[:, 0:128], ident[:, :], ident[:, :])

        # masks live flat-reshaped on 80 partitions: partition k*4+b holds
        # fg[k, b*4096:(b+1)*4096] (k<16), partition 64+k*4+b holds the same
        # for bg. A [80,128] PE transpose then yields, per 128-pixel chunk,
        # all 20 mask rows as stride-4 columns — gathered by the matmul's
        # weight AP. Keeps every DMA at 16KB/partition on many SBUF ports.
        NSEG = N // 4  # 4096
        m80 = singles.tile([80, NSEG], F32)

        for rep in range(reps):
            # ---- masks: flat load, transpose on PE, threshold from PSUM ----
            nc.sync.dma_start(
                out=m80[0:64, :],
                in_=fg[:, :].rearrange("k (b m) -> (k b) m", b=4),
            )
            nc.sync.dma_start(
                out=m80[64:80, :],
                in_=bg[:, :].rearrange("k (b m) -> (k b) m", b=4),
            )

            # wT free layout: (j, p) with j = chunk-within-block (32), p = k*4+b (80)
            wT = per_rep.tile([128, 32 * 80], F16)
            wT_v = wT[:, :].rearrange("q (j k b) -> q j b k", j=32, k=K20, b=4)

            acc = psA.tile([K20, C], F32)
            cnt = psW.tile([K20, 1], F32)

            TPB = 4  # mask transposes per PSUM tile
            for batch in range(32 // TPB):
                mt = psM.tile([128, TPB * 80], F32)
                for i in range(TPB):
                    j = batch * TPB + i
                    nc.tensor.transpose(
                        mt[:, i * 80 : (i + 1) * 80],
                        m80[:, j * CH : (j + 1) * CH],
                        ident[0:80, 0:80],
                    )
                lo, hi = batch * TPB * 80, (batch + 1) * TPB * 80
                # threshold in fp32 straight out of PSUM; 0/1 is exact in fp16
                nc.vector.tensor_scalar(
                    out=wT[:, lo:hi],
                    in0=mt[:, :],
                    scalar1=0.5,
                    scalar2=None,
                    op0=mybir.AluOpType.is_gt,
                )

            # ---- main loop: stream f, transpose on PE, matmul-accumulate ----
            for sc in range(SC):
                ft = fpool.tile([128, 2, NSC], F16)
                for cb in range(2):
                    # SWDGE casts f32 -> fp16 inline during the load
                    nc.gpsimd.dma_start(
                        out=ft[:, cb, :],
                        in_=fmap[cb * 128 : (cb + 1) * 128, sc * NSC : (sc + 1) * NSC],
                    )
                for jp in range(CPS // 2):  # chunks in pairs
                    pt = psT.tile([128, 2, C], F16)
                    for u in range(2):
                        j = jp * 2 + u
                        for cb in range(2):
                            nc.tensor.transpose(
                                pt[:, u, cb * 128 : (cb + 1) * 128],
                                ft[:, cb, j * CH : (j + 1) * CH],
                                ident16[:, :],
                            )
                    rhs = rhspool.tile([128, 2, C], F16)
                    if jp % 2 == 0:
                        nc.vector.tensor_copy(rhs[:, :, :], pt[:, :, :])
                    else:
                        nc.scalar.copy(rhs[:, :, :], pt[:, :, :])
                    for u in range(2):
                        g = sc * CPS + jp * 2 + u
                        blk, j = g // 32, g % 32
                        w_g = wT_v[:, j, blk, :]
                        first, last = g == 0, g == NCHUNK - 1
                        nc.tensor.matmul(
                            acc[:, :],
                            lhsT=w_g,
                            rhs=rhs[:, u, :],
                            start=first,
                            stop=last,
                        )
                        nc.tensor.matmul(
                            cnt[:, :],
                            lhsT=w_g,
                            rhs=ones[:, :],
                            start=first,
                            stop=last,
                        )

            # ---- epilogue: divide by max(cnt, 1) ----
            res = per_rep.tile([K20, C], F32)
            csb = per_rep.tile([K20, 1], F32)
            rec = per_rep.tile([K20, 1], F32)
            nc.vector.tensor_copy(res[:, :], acc[:, :])
            nc.vector.tensor_copy(csb[:, :], cnt[:, :])
            nc.vector.tensor_scalar_max(csb, csb, 1.0)
            nc.vector.reciprocal(rec, csb)
            nc.vector.tensor_scalar_mul(res, res, rec[:, :])
            nc.sync.dma_start(out=fg_out[:, :], in_=res[0:I_FG, :])
            nc.sync.dma_start(out=bg_out[:, :], in_=res[I_FG:K20, :])

    nc.compile()
    return nc


def _get_nc(reps=1):
    if reps not in _nc_cache:
        _nc_cache[reps] = _build_nc(reps)
    return _nc_cache[reps]


def _make_runner(reps=1, donate=True):
    """Build a cached callable (fmap, fg, bg full np arrays) -> jax outputs.
    Keeps device-resident inputs and one compiled executable so repeated calls
    measure device time + small dispatch overhead only.  With donate=False the
    zero output-seed buffers stay device-resident too, so back-to-back calls
    have no host transfers at all and can pipeline asynchronously (pass
    block=False to skip the final block_until_ready)."""
    key = (reps, donate)
    if key in _runner_cache:
        return _runner_cache[key]

    import jax
    from jax.sharding import Mesh, PartitionSpec
    from jax.experimental.shard_map import shard_map
    from concourse import bass2jax

    bass2jax.install_neuronx_cc_hook()
    nc = _get_nc(reps)

    in_names = ["fmap", "fg", "bg"]
    out_names = ["fg_out", "bg_out"]
    out_avals = [
        jax.core.ShapedArray((I_FG, C), np.float32),
        jax.core.ShapedArray((Q_BG, C), np.float32),
    ]
    all_in_names = in_names + out_names
    if nc.partition_id_tensor is not None:
        all_in_names.append(nc.partition_id_tensor.name)

    def _body(*args):
        operands = list(args)
        if nc.partition_id_tensor is not None:
            operands.append(bass2jax.partition_id_tensor())
        outs = bass2jax._bass_exec_p.bind(
            *operands,
            out_avals=tuple(out_avals),
            in_names=tuple(all_in_names),
            out_names=tuple(out_names),
            lowering_input_output_aliases=(),
            sim_require_finite=True,
            sim_require_nnan=True,
            nc=nc,
        )
        return tuple(outs)

    devices = jax.devices()[:B]
    mesh = Mesh(np.asarray(devices), ("core",))
    n_in = len(in_names) + len(out_names)
    donate_nums = tuple(range(len(in_names), n_in)) if donate else ()
    sharded = jax.jit(
        shard_map(
            _body,
            mesh=mesh,
            in_specs=(PartitionSpec("core"),) * n_in,
            out_specs=(PartitionSpec("core"),) * len(out_names),
            check_rep=False,
        ),
        donate_argnums=donate_nums,
        keep_unused=True,
    )

    state = {}

    def runner(fmap, fg_mask, bg_mask, block=True):
        key = (fmap.ctypes.data, fg_mask.ctypes.data, bg_mask.ctypes.data)
        if state.get("key") != key:
            state["key"] = key
            state["in"] = [
                jax.device_put(fmap.reshape(B * C, N)),
                jax.device_put(fg_mask.reshape(B * I_FG, N)),
                jax.device_put(bg_mask.reshape(B * Q_BG, N)),
            ]
            if not donate:
                state["zeros"] = [
                    jax.device_put(np.zeros((B * I_FG, C), np.float32)),
                    jax.device_put(np.zeros((B * Q_BG, C), np.float32)),
                ]
        if donate:
            zeros = [
                np.zeros((B * I_FG, C), np.float32),
                np.zeros((B * Q_BG, C), np.float32),
            ]
        else:
            zeros = state["zeros"]
        outs = sharded(*state["in"], *zeros)
        if block:
            jax.block_until_ready(outs)
        return outs

    _runner_cache[key] = runner
    return runner


def _make_chain_runner(n_chain):
    """One jit that executes the bass NEFF n_chain times, chaining each call's
    outputs into the next call's output-seed operands (data dependency defeats
    CSE, so XLA runs them sequentially). One dispatch total — the per-call
    tunnel overhead cancels when comparing different n_chain."""
    if ("chain", n_chain) in _runner_cache:
        return _runner_cache[("chain", n_chain)]

    import jax
    from jax.sharding import Mesh, PartitionSpec
    from jax.experimental.shard_map import shard_map
    from concourse import bass2jax

    bass2jax.install_neuronx_cc_hook()
    nc = _get_nc(1)

    in_names = ["fmap", "fg", "bg"]
    out_names = ["fg_out", "bg_out"]
    out_avals = [
        jax.core.ShapedArray((I_FG, C), np.float32),
        jax.core.ShapedArray((Q_BG, C), np.float32),
    ]
    all_in_names = in_names + out_names
    if nc.partition_id_tensor is not None:
        all_in_names.append(nc.partition_id_tensor.name)

    def _body(*args):
        fmap_a, fg_a, bg_a, z0, z1 = args
        for _ in range(n_chain):
            operands = [fmap_a, fg_a, bg_a, z0, z1]
            if nc.partition_id_tensor is not None:
                operands.append(bass2jax.partition_id_tensor())
            z0, z1 = bass2jax._bass_exec_p.bind(
                *operands,
                out_avals=tuple(out_avals),
                in_names=tuple(all_in_names),
                out_names=tuple(out_names),
                lowering_input_output_aliases=(),
                sim_require_finite=True,
                sim_require_nnan=True,
                nc=nc,
            )
        return z0, z1

    devices = jax.devices()[:B]
    mesh = Mesh(np.asarray(devices), ("core",))
    sharded = jax.jit(
        shard_map(
            _body,
            mesh=mesh,
            in_specs=(PartitionSpec("core"),) * 5,
            out_specs=(PartitionSpec("core"),) * 2,
            check_rep=False,
        ),
        keep_unused=True,
    )

    state = {}

    def runner(fmap, fg_mask, bg_mask, block=True):
        key = (fmap.ctypes.data,)
        if state.get("key") != key:
            import jax as _jax
            state["key"] = key
            state["in"] = [
                _jax.device_put(fmap.reshape(B * C, N)),
                _jax.device_put(fg_mask.reshape(B * I_FG, N)),
                _jax.device_put(bg_mask.reshape(B * Q_BG, N)),
                _jax.device_put(np.zeros((B * I_FG, C), np.float32)),
                _jax.device_put(np.zeros((B * Q_BG, C), np.float32)),
            ]
        import jax as _jax
        outs = sharded(*state["in"])
        if block:
            _jax.block_until_ready(outs)
        return outs

    _runner_cache[("chain", n_chain)] = runner
    return runner


def run(fmap, fg_mask, bg_mask, **spmd_kwargs):
    """Run on 8 NeuronCores via run_bass_kernel_spmd; returns
    (fg_init, bg_init, BassKernelResults)."""
    from concourse.bass_utils import run_bass_kernel_spmd

    fmap = np.ascontiguousarray(np.asarray(fmap, dtype=np.float32))
    fg_mask = np.ascontiguousarray(np.asarray(fg_mask, dtype=np.float32))
    bg_mask = np.ascontiguousarray(np.asarray(bg_mask, dtype=np.float32))
    assert fmap.shape == (B, C, H, W)
    assert fg_mask.shape == (B, I_FG, H, W)
    assert bg_mask.shape == (B, Q_BG, H, W)

    in_maps = [
        {
            "fmap": fmap[b].reshape(C, N),
            "fg": fg_mask[b].reshape(I_FG, N),
            "bg": bg_mask[b].reshape(Q_BG, N),
        }
        for b in range(B)
    ]
    nc = _get_nc(1)
    res = run_bass_kernel_spmd(nc, in_maps, core_ids=list(range(B)), **spmd_kwargs)
    fg_init = np.stack([r["fg_out"] for r in res.results]).astype(np.float32)
    bg_init = np.stack([r["bg_out"] for r in res.results]).astype(np.float32)
    return fg_init, bg_init, res


def kernel(fmap, fg_mask, bg_mask):
    fg_init, bg_init, _ = run(fmap, fg_mask, bg_mask)
    return fg_init, bg_init


# revision 23
# speedup vs baseline: 11.2087x; 1.4058x over previous
"""Trainium2 Bass kernel for masked average pooling (AvgPoolingInitializer).

Computes, per example b:
    fg_init[b,i,c] = mean of fmap[b,c,:,:] over pixels where fg_mask[b,i,:,:] > 0.5
    bg_init[b,q,c] = mean of fmap[b,c,:,:] over pixels where bg_mask[b,q,:,:] > 0.5
                     (zeros if the mask is empty)

Sharding: pure data parallel, one example per NeuronCore (B=8 examples, 8 cores).

Per-core algorithm (n = H*W = 16384 flattened pixels):
  - masks [20, n] are PE-transposed in chunks of 128, then thresholded in fp32
    straight out of PSUM (>0.5 -> 1.0/0.0) into an fp16 wT slab [n-chunk, 20].
  - fmap [256, n] is streamed in natural layout, each [128c, 128n] tile is
    PE-transposed (fp32, exact) into PSUM, copied+rounded to fp16 in SBUF
    (alternating DVE/ACT), and contracted against wT on the PE:
        acc[20, 256] += wT_chunk.T @ fT_chunk   (fp16 in, fp32 accumulate)
        cnt[20, 1]   += wT_chunk.T @ ones       (fp32, exact)
  - epilogue: out = acc * 1/max(cnt, 1), DMA to fg_out/bg_out.

`reps` repeats the whole per-example pipeline inside one NEFF; used by the
timing harness to measure per-iteration device time as a slope, cancelling
the constant dispatch/tunnel overhead.
"""

import numpy as np

B = 8
C = 256
H = W = 128
N = H * W          # 16384
I_FG = 16
Q_BG = 4
K20 = I_FG + Q_BG  # 20 combined mask rows
CH = 128           # contraction chunk width (PE partition dim)
NCHUNK = N // CH   # 128 chunks
SC = 4             # superchunks (DMA granularity: 2 MiB per c-block)
NSC = N // SC      # 2048 columns per superchunk
CPS = NSC // CH    # 16 chunks per superchunk
MB = 8             # mask chunks batched per PSUM tile

_nc_cache = {}
_runner_cache = {}


def _build_nc(reps=1, cast_f16=True):
    import concourse.bacc as bacc
    import concourse.tile as tile
    import concourse.mybir as mybir
    from concourse.masks import make_identity
    from contextlib import ExitStack

    F16 = mybir.dt.float16
    F32 = mybir.dt.float32

    nc = bacc.Bacc("TRN2")
    fmap = nc.dram_tensor("fmap", [C, N], F32, kind="ExternalInput")
    fg = nc.dram_tensor("fg", [I_FG, N], F32, kind="ExternalInput")
    bg = nc.dram_tensor("bg", [Q_BG, N], F32, kind="ExternalInput")
    fg_out = nc.dram_tensor("fg_out", [I_FG, C], F32, kind="ExternalOutput")
    bg_out = nc.dram_tensor("bg_out", [Q_BG, C], F32, kind="ExternalOutput")

    with tile.TileContext(nc) as tc, ExitStack() as ctx:
        singles = ctx.enter_context(tc.tile_pool(name="singles", bufs=1))
        per_rep = ctx.enter_context(tc.tile_pool(name="per_rep", bufs=min(2, reps)))
        fpool = ctx.enter_context(tc.tile_pool(name="fpool", bufs=4))
        rhspool = ctx.enter_context(tc.tile_pool(name="rhs", bufs=6))
        psA = ctx.enter_context(tc.tile_pool(name="psA", bufs=1, space="PSUM"))
        psT = ctx.enter_context(tc.tile_pool(name="psT", bufs=3, space="PSUM"))
        psM = ctx.enter_context(tc.tile_pool(name="psM", bufs=2, space="PSUM"))
        psW = ctx.enter_context(tc.tile_pool(name="psW", bufs=1, space="PSUM"))

        ident = singles.tile([128, 128], F32)
        make_identity(nc, ident)
        ident16 = singles.tile([128, 128], F16)
        nc.vector.tensor_copy(ident16[:, :], ident[:, :])
        ones = singles.tile([128, 1], F16)
        nc.vector.memset(ones, 1.0)

        # dummy transpose so the PE's vector clock covers the identity's
        # producer (gpsimd) before the mask DMAs land — keeps every matmul's
        # LDWEIGHTS at <=1 sync wait after bacc's event-semaphore split.
        warm = psW.tile([128, MB * K20], F32)
        nc.tensor.transpose(warm[:, 0:128], ident[:, :], ident[:, :])

        # masks live flat-reshaped on 80 partitions: partition k*4+b holds
        # fg[k, b*4096:(b+1)*4096] (k<16), partition 64+k*4+b holds the same
        # for bg. A [80,128] PE transpose then yields, per 128-pixel chunk,
        # all 20 mask rows as stride-4 columns — gathered by the matmul's
        # weight AP. Keeps every DMA at 16KB/partition on many SBUF ports.
        NSEG = N // 4  # 4096
        m80 = singles.tile([80, NSEG], F32)

        for rep in range(reps):
            # ---- masks: flat load, transpose on PE, threshold from PSUM ----
            nc.sync.dma_start(
                out=m80[0:64, :],
                in_=fg[:, :].rearrange("k (b m) -> (k b) m", b=4),
            )
            nc.scalar.dma_start(
                out=m80[64:80, :],
                in_=bg[:, :].rearrange("k (b m) -> (k b) m", b=4),
            )

            # wT free layout: (j, p) with j = chunk-within-block (32), p = k*4+b (80)
            wT = per_rep.tile([128, 32 * 80], F16)
            wT_v = wT[:, :].rearrange("q (j k b) -> q j b k", j=32, k=K20, b=4)

            acc = psA.tile([K20, C], F32)
            cnt = psW.tile([K20, 1], F32)

            TPB = 4  # mask transposes per PSUM tile
            for batch in range(32 // TPB):
                mt = psM.tile([128, TPB * 80], F32)
                for i in range(TPB):
                    j = batch * TPB + i
                    nc.tensor.transpose(
                        mt[:, i * 80 : (i + 1) * 80],
                        m80[:, j * CH : (j + 1) * CH],
                        ident[0:80, 0:80],
                    )
                lo, hi = batch * TPB * 80, (batch + 1) * TPB * 80
                # threshold in fp32 straight out of PSUM; 0/1 is exact in fp16
                nc.vector.tensor_scalar(
                    out=wT[:, lo:hi],
                    in0=mt[:, :],
                    scalar1=0.5,
                    scalar2=None,
                    op0=mybir.AluOpType.is_gt,
                )

            # ---- main loop: stream f, transpose on PE, matmul-accumulate ----
            for sc in range(SC):
                ft = fpool.tile([128, 2, NSC], F16 if cast_f16 else F32)
                for cb in range(2):
                    # SWDGE casts f32 -> fp16 inline during the load;
                    # the f32 variant splits loads across both DGE rings
                    eng = nc.gpsimd if (cast_f16 or cb == 1) else nc.sync
                    eng.dma_start(
                        out=ft[:, cb, :],
                        in_=fmap[cb * 128 : (cb + 1) * 128, sc * NSC : (sc + 1) * NSC],
                    )
                for jp in range(CPS // 2):  # chunks in pairs
                    pt = psT.tile([128, 2, C], F16 if cast_f16 else F32)
                    for u in range(2):
                        j = jp * 2 + u
                        for cb in range(2):
                            nc.tensor.transpose(
                                pt[:, u, cb * 128 : (cb + 1) * 128],
                                ft[:, cb, j * CH : (j + 1) * CH],
                                ident16[:, :] if cast_f16 else ident[:, :],
                            )
                    rhs = rhspool.tile([128, 2, C], F16)
                    if jp % 2 == 0:
                        nc.vector.tensor_copy(rhs[:, :, :], pt[:, :, :])
                    else:
                        nc.scalar.copy(rhs[:, :, :], pt[:, :, :])
                    for u in range(2):
                        g = sc * CPS + jp * 2 + u
                        blk, j = g // 32, g % 32
                        w_g = wT_v[:, j, blk, :]
                        first, last = g == 0, g == NCHUNK - 1
                        nc.tensor.matmul(
                            acc[:, :],
                            lhsT=w_g,
                            rhs=rhs[:, u, :],
                            start=first,
                            stop=last,
                        )
                        nc.tensor.matmul(
                            cnt[:, :],
                            lhsT=w_g,
                            rhs=ones[:, :],
                            start=first,
                            stop=last,
                        )

            # ---- epilogue: divide by max(cnt, 1) ----
            res = per_rep.tile([K20, C], F32)
            csb = per_rep.tile([K20, 1], F32)
            rec = per_rep.tile([K20, 1], F32)
            nc.vector.tensor_copy(res[:, :], acc[:, :])
            nc.vector.tensor_copy(csb[:, :], cnt[:, :])
            nc.vector.tensor_scalar_max(csb, csb, 1.0)
            nc.vector.reciprocal(rec, csb)
            nc.vector.tensor_scalar_mul(res, res, rec[:, :])
            nc.sync.dma_start(out=fg_out[:, :], in_=res[0:I_FG, :])
            nc.sync.dma_start(out=bg_out[:, :], in_=res[I_FG:K20, :])

    nc.compile()
    return nc


def _get_nc(reps=1, cast_f16=True):
    key = (reps, cast_f16)
    if key not in _nc_cache:
        _nc_cache[key] = _build_nc(reps, cast_f16)
    return _nc_cache[key]


def _make_runner(reps=1, donate=True):
    """Build a cached callable (fmap, fg, bg full np arrays) -> jax outputs.
    Keeps device-resident inputs and one compiled executable so repeated calls
    measure device time + small dispatch overhead only.  With donate=False the
    zero output-seed buffers stay device-resident too, so back-to-back calls
    have no host transfers at all and can pipeline asynchronously (pass
    block=False to skip the final block_until_ready)."""
    key = (reps, donate)
    if key in _runner_cache:
        return _runner_cache[key]

    import jax
    from jax.sharding import Mesh, PartitionSpec
    from jax.experimental.shard_map import shard_map
    from concourse import bass2jax

    bass2jax.install_neuronx_cc_hook()
    nc = _get_nc(reps)

    in_names = ["fmap", "fg", "bg"]
    out_names = ["fg_out", "bg_out"]
    out_avals = [
        jax.core.ShapedArray((I_FG, C), np.float32),
        jax.core.ShapedArray((Q_BG, C), np.float32),
    ]
    all_in_names = in_names + out_names
    if nc.partition_id_tensor is not None:
        all_in_names.append(nc.partition_id_tensor.name)

    def _body(*args):
        operands = list(args)
        if nc.partition_id_tensor is not None:
            operands.append(bass2jax.partition_id_tensor())
        outs = bass2jax._bass_exec_p.bind(
            *operands,
            out_avals=tuple(out_avals),
            in_names=tuple(all_in_names),
            out_names=tuple(out_names),
            lowering_input_output_aliases=(),
            sim_require_finite=True,
            sim_require_nnan=True,
            nc=nc,
        )
        return tuple(outs)

    devices = jax.devices()[:B]
    mesh = Mesh(np.asarray(devices), ("core",))
    n_in = len(in_names) + len(out_names)
    donate_nums = tuple(range(len(in_names), n_in)) if donate else ()
    sharded = jax.jit(
        shard_map(
            _body,
            mesh=mesh,
            in_specs=(PartitionSpec("core"),) * n_in,
            out_specs=(PartitionSpec("core"),) * len(out_names),
            check_rep=False,
        ),
        donate_argnums=donate_nums,
        keep_unused=True,
    )

    state = {}

    def runner(fmap, fg_mask, bg_mask, block=True):
        key = (fmap.ctypes.data, fg_mask.ctypes.data, bg_mask.ctypes.data)
        if state.get("key") != key:
            state["key"] = key
            state["in"] = [
                jax.device_put(fmap.reshape(B * C, N)),
                jax.device_put(fg_mask.reshape(B * I_FG, N)),
                jax.device_put(bg_mask.reshape(B * Q_BG, N)),
            ]
            if not donate:
                state["zeros"] = [
                    jax.device_put(np.zeros((B * I_FG, C), np.float32)),
                    jax.device_put(np.zeros((B * Q_BG, C), np.float32)),
                ]
        if donate:
            zeros = [
                np.zeros((B * I_FG, C), np.float32),
                np.zeros((B * Q_BG, C), np.float32),
            ]
        else:
            zeros = state["zeros"]
        outs = sharded(*state["in"], *zeros)
        if block:
            jax.block_until_ready(outs)
        return outs

    _runner_cache[key] = runner
    return runner


def _make_chain_runner(n_chain):
    """One jit that executes the bass NEFF n_chain times, chaining each call's
    outputs into the next call's output-seed operands (data dependency defeats
    CSE, so XLA runs them sequentially). One dispatch total — the per-call
    tunnel overhead cancels when comparing different n_chain."""
    if ("chain", n_chain) in _runner_cache:
        return _runner_cache[("chain", n_chain)]

    import jax
    from jax.sharding import Mesh, PartitionSpec
    from jax.experimental.shard_map import shard_map
    from concourse import bass2jax

    bass2jax.install_neuronx_cc_hook()
    nc = _get_nc(1)

    in_names = ["fmap", "fg", "bg"]
    out_names = ["fg_out", "bg_out"]
    out_avals = [
        jax.core.ShapedArray((I_FG, C), np.float32),
        jax.core.ShapedArray((Q_BG, C), np.float32),
    ]
    all_in_names = in_names + out_names
    if nc.partition_id_tensor is not None:
        all_in_names.append(nc.partition_id_tensor.name)

    def _body(*args):
        fmap_a, fg_a, bg_a, z0, z1 = args
        for _ in range(n_chain):
            operands = [fmap_a, fg_a, bg_a, z0, z1]
            if nc.partition_id_tensor is not None:
                operands.append(bass2jax.partition_id_tensor())
            z0, z1 = bass2jax._bass_exec_p.bind(
                *operands,
                out_avals=tuple(out_avals),
                in_names=tuple(all_in_names),
                out_names=tuple(out_names),
                lowering_input_output_aliases=(),
                sim_require_finite=True,
                sim_require_nnan=True,
                nc=nc,
            )
        return z0, z1

    devices = jax.devices()[:B]
    mesh = Mesh(np.asarray(devices), ("core",))
    sharded = jax.jit(
        shard_map(
            _body,
            mesh=mesh,
            in_specs=(PartitionSpec("core"),) * 5,
            out_specs=(PartitionSpec("core"),) * 2,
            check_rep=False,
        ),
        keep_unused=True,
    )

    state = {}

    def runner(fmap, fg_mask, bg_mask, block=True):
        key = (fmap.ctypes.data,)
        if state.get("key") != key:
            import jax as _jax
            state["key"] = key
            state["in"] = [
                _jax.device_put(fmap.reshape(B * C, N)),
                _jax.device_put(fg_mask.reshape(B * I_FG, N)),
                _jax.device_put(bg_mask.reshape(B * Q_BG, N)),
                _jax.device_put(np.zeros((B * I_FG, C), np.float32)),
                _jax.device_put(np.zeros((B * Q_BG, C), np.float32)),
            ]
        import jax as _jax
        outs = sharded(*state["in"])
        if block:
            _jax.block_until_ready(outs)
        return outs

    _runner_cache[("chain", n_chain)] = runner
    return runner


def run(fmap, fg_mask, bg_mask, **spmd_kwargs):
    """Run on 8 NeuronCores via run_bass_kernel_spmd; returns
    (fg_init, bg_init, BassKernelResults)."""
    from concourse.bass_utils import run_bass_kernel_spmd

    fmap = np.ascontiguousarray(np.asarray(fmap, dtype=np.float32))
    fg_mask = np.ascontiguousarray(np.asarray(fg_mask, dtype=np.float32))
    bg_mask = np.ascontiguousarray(np.asarray(bg_mask, dtype=np.float32))
    assert fmap.shape == (B, C, H, W)
    assert fg_mask.shape == (B, I_FG, H, W)
    assert bg_mask.shape == (B, Q_BG, H, W)

    in_maps = [
        {
            "fmap": fmap[b].reshape(C, N),
            "fg": fg_mask[b].reshape(I_FG, N),
            "bg": bg_mask[b].reshape(Q_BG, N),
        }
        for b in range(B)
    ]
    nc = _get_nc(1)
    res = run_bass_kernel_spmd(nc, in_maps, core_ids=list(range(B)), **spmd_kwargs)
    fg_init = np.stack([r["fg_out"] for r in res.results]).astype(np.float32)
    bg_init = np.stack([r["bg_out"] for r in res.results]).astype(np.float32)
    return fg_init, bg_init, res


def kernel(fmap, fg_mask, bg_mask):
    fg_init, bg_init, _ = run(fmap, fg_mask, bg_mask)
    return fg_init, bg_init


# revision 24
# speedup vs baseline: 17.6257x; 1.5725x over previous
"""Trainium2 Bass kernel for masked average pooling (AvgPoolingInitializer).

Computes, per example b:
    fg_init[b,i,c] = mean of fmap[b,c,:,:] over pixels where fg_mask[b,i,:,:] > 0.5
    bg_init[b,q,c] = mean of fmap[b,c,:,:] over pixels where bg_mask[b,q,:,:] > 0.5
                     (zeros if the mask is empty)

Sharding: pure data parallel, one example per NeuronCore (B=8 examples, 8 cores).

Per-core algorithm (n = H*W = 16384 flattened pixels):
  - masks [20, n] are PE-transposed in chunks of 128, then thresholded in fp32
    straight out of PSUM (>0.5 -> 1.0/0.0) into an fp16 wT slab [n-chunk, 20].
  - fmap [256, n] is streamed in natural layout, each [128c, 128n] tile is
    PE-transposed (fp32, exact) into PSUM, copied+rounded to fp16 in SBUF
    (alternating DVE/ACT), and contracted against wT on the PE:
        acc[20, 256] += wT_chunk.T @ fT_chunk   (fp16 in, fp32 accumulate)
        cnt[20, 1]   += wT_chunk.T @ ones       (fp32, exact)
  - epilogue: out = acc * 1/max(cnt, 1), DMA to fg_out/bg_out.

`reps` repeats the whole per-example pipeline inside one NEFF; used by the
timing harness to measure per-iteration device time as a slope, cancelling
the constant dispatch/tunnel overhead.
"""

import numpy as np

B = 8
C = 256
H = W = 128
N = H * W          # 16384
I_FG = 16
Q_BG = 4
K20 = I_FG + Q_BG  # 20 combined mask rows
CH = 128           # contraction chunk width (PE partition dim)
NCHUNK = N // CH   # 128 chunks
SC = 4             # superchunks (DMA granularity: 2 MiB per c-block)
NSC = N // SC      # 2048 columns per superchunk
CPS = NSC // CH    # 16 chunks per superchunk
MB = 8             # mask chunks batched per PSUM tile

_nc_cache = {}
_runner_cache = {}


def _build_nc(reps=1, cast_f16=True):
    import concourse.bacc as bacc
    import concourse.tile as tile
    import concourse.mybir as mybir
    from concourse.masks import make_identity
    from contextlib import ExitStack

    F16 = mybir.dt.float16
    F32 = mybir.dt.float32

    nc = bacc.Bacc("TRN2")
    fmap = nc.dram_tensor("fmap", [C, N], F32, kind="ExternalInput")
    fg = nc.dram_tensor("fg", [I_FG, N], F32, kind="ExternalInput")
    bg = nc.dram_tensor("bg", [Q_BG, N], F32, kind="ExternalInput")
    fg_out = nc.dram_tensor("fg_out", [I_FG, C], F32, kind="ExternalOutput")
    bg_out = nc.dram_tensor("bg_out", [Q_BG, C], F32, kind="ExternalOutput")

    with tile.TileContext(nc) as tc, ExitStack() as ctx:
        singles = ctx.enter_context(tc.tile_pool(name="singles", bufs=1))
        per_rep = ctx.enter_context(tc.tile_pool(name="per_rep", bufs=min(2, reps)))
        fpool = ctx.enter_context(tc.tile_pool(name="fpool", bufs=4))
        rhspool = ctx.enter_context(tc.tile_pool(name="rhs", bufs=6))
        psA = ctx.enter_context(tc.tile_pool(name="psA", bufs=min(2, reps), space="PSUM"))
        psT = ctx.enter_context(tc.tile_pool(name="psT", bufs=3 if reps == 1 else 2, space="PSUM"))
        psM = ctx.enter_context(tc.tile_pool(name="psM", bufs=2, space="PSUM"))
        psW = ctx.enter_context(tc.tile_pool(name="psW", bufs=1, space="PSUM"))

        ident = singles.tile([128, 128], F32)
        make_identity(nc, ident)
        ident16 = singles.tile([128, 128], F16)
        nc.vector.tensor_copy(ident16[:, :], ident[:, :])
        ones = singles.tile([128, 1], F16)
        nc.vector.memset(ones, 1.0)

        # dummy transpose so the PE's vector clock covers the identity's
        # producer (gpsimd) before the mask DMAs land — keeps every matmul's
        # LDWEIGHTS at <=1 sync wait after bacc's event-semaphore split.
        warm = psW.tile([128, MB * K20], F32)
        nc.tensor.transpose(warm[:, 0:128], ident[:, :], ident[:, :])

        # masks live flat-reshaped on 80 partitions: partition k*4+b holds
        # fg[k, b*4096:(b+1)*4096] (k<16), partition 64+k*4+b holds the same
        # for bg. A [80,128] PE transpose then yields, per 128-pixel chunk,
        # all 20 mask rows as stride-4 columns — gathered by the matmul's
        # weight AP. Keeps every DMA at 16KB/partition on many SBUF ports.
        NSEG = N // 4  # 4096

        for rep in range(reps):
            m80 = per_rep.tile([80, NSEG], F32)
            # ---- masks: flat load, transpose on PE, threshold from PSUM ----
            nc.sync.dma_start(
                out=m80[0:64, :],
                in_=fg[:, :].rearrange("k (b m) -> (k b) m", b=4),
            )
            nc.scalar.dma_start(
                out=m80[64:80, :],
                in_=bg[:, :].rearrange("k (b m) -> (k b) m", b=4),
            )

            # wT free layout: (j, p) with j = chunk-within-block (32), p = k*4+b (80)
            wT = per_rep.tile([128, 32 * 80], F16)
            wT_v = wT[:, :].rearrange("q (j k b) -> q j b k", j=32, k=K20, b=4)

            acc = psA.tile([K20, C], F32)
            cnt = psW.tile([K20, 1], F32)

            TPB = 4  # mask transposes per PSUM tile
            for batch in range(32 // TPB):
                mt = psM.tile([128, TPB * 80], F32)
                for i in range(TPB):
                    j = batch * TPB + i
                    nc.tensor.transpose(
                        mt[:, i * 80 : (i + 1) * 80],
                        m80[:, j * CH : (j + 1) * CH],
                        ident[0:80, 0:80],
                    )
                lo, hi = batch * TPB * 80, (batch + 1) * TPB * 80
                # threshold in fp32 straight out of PSUM; 0/1 is exact in fp16
                nc.vector.tensor_scalar(
                    out=wT[:, lo:hi],
                    in0=mt[:, :],
                    scalar1=0.5,
                    scalar2=None,
                    op0=mybir.AluOpType.is_gt,
                )

            # ---- main loop: stream f, transpose on PE, matmul-accumulate ----
            for sc in range(SC):
                ft = fpool.tile([128, 2, NSC], F16 if cast_f16 else F32)
                for cb in range(2):
                    # SWDGE casts f32 -> fp16 inline during the load;
                    # the f32 variant splits loads across both DGE rings
                    eng = nc.gpsimd if (cast_f16 or cb == 1) else nc.sync
                    eng.dma_start(
                        out=ft[:, cb, :],
                        in_=fmap[cb * 128 : (cb + 1) * 128, sc * NSC : (sc + 1) * NSC],
                    )
                for jp in range(CPS // 2):  # chunks in pairs
                    pt = psT.tile([128, 2, C], F16 if cast_f16 else F32)
                    for u in range(2):
                        j = jp * 2 + u
                        for cb in range(2):
                            nc.tensor.transpose(
                                pt[:, u, cb * 128 : (cb + 1) * 128],
                                ft[:, cb, j * CH : (j + 1) * CH],
                                ident16[:, :] if cast_f16 else ident[:, :],
                            )
                    rhs = rhspool.tile([128, 2, C], F16)
                    if jp % 2 == 0:
                        nc.vector.tensor_copy(rhs[:, :, :], pt[:, :, :])
                    else:
                        nc.scalar.copy(rhs[:, :, :], pt[:, :, :])
                    for u in range(2):
                        g = sc * CPS + jp * 2 + u
                        blk, j = g // 32, g % 32
                        w_g = wT_v[:, j, blk, :]
                        first, last = g == 0, g == NCHUNK - 1
                        nc.tensor.matmul(
                            acc[:, :],
                            lhsT=w_g,
                            rhs=rhs[:, u, :],
                            start=first,
                            stop=last,
                        )
                        nc.tensor.matmul(
                            cnt[:, :],
                            lhsT=w_g,
                            rhs=ones[:, :],
                            start=first,
                            stop=last,
                        )

            # ---- epilogue: divide by max(cnt, 1) ----
            res = per_rep.tile([K20, C], F32)
            csb = per_rep.tile([K20, 1], F32)
            rec = per_rep.tile([K20, 1], F32)
            nc.vector.tensor_copy(res[:, :], acc[:, :])
            nc.vector.tensor_copy(csb[:, :], cnt[:, :])
            nc.vector.tensor_scalar_max(csb, csb, 1.0)
            nc.vector.reciprocal(rec, csb)
            nc.vector.tensor_scalar_mul(res, res, rec[:, :])
            nc.sync.dma_start(out=fg_out[:, :], in_=res[0:I_FG, :])
            nc.sync.dma_start(out=bg_out[:, :], in_=res[I_FG:K20, :])

    nc.compile()
    return nc


def _get_nc(reps=1, cast_f16=True):
    key = (reps, cast_f16)
    if key not in _nc_cache:
        _nc_cache[key] = _build_nc(reps, cast_f16)
    return _nc_cache[key]


def _make_runner(reps=1, donate=True):
    """Build a cached callable (fmap, fg, bg full np arrays) -> jax outputs.
    Keeps device-resident inputs and one compiled executable so repeated calls
    measure device time + small dispatch overhead only.  With donate=False the
    zero output-seed buffers stay device-resident too, so back-to-back calls
    have no host transfers at all and can pipeline asynchronously (pass
    block=False to skip the final block_until_ready)."""
    key = (reps, donate)
    if key in _runner_cache:
        return _runner_cache[key]

    import jax
    from jax.sharding import Mesh, PartitionSpec
    from jax.experimental.shard_map import shard_map
    from concourse import bass2jax

    bass2jax.install_neuronx_cc_hook()
    nc = _get_nc(reps)

    in_names = ["fmap", "fg", "bg"]
    out_names = ["fg_out", "bg_out"]
    out_avals = [
        jax.core.ShapedArray((I_FG, C), np.float32),
        jax.core.ShapedArray((Q_BG, C), np.float32),
    ]
    all_in_names = in_names + out_names
    if nc.partition_id_tensor is not None:
        all_in_names.append(nc.partition_id_tensor.name)

    def _body(*args):
        operands = list(args)
        if nc.partition_id_tensor is not None:
            operands.append(bass2jax.partition_id_tensor())
        outs = bass2jax._bass_exec_p.bind(
            *operands,
            out_avals=tuple(out_avals),
            in_names=tuple(all_in_names),
            out_names=tuple(out_names),
            lowering_input_output_aliases=(),
            sim_require_finite=True,
            sim_require_nnan=True,
            nc=nc,
        )
        return tuple(outs)

    devices = jax.devices()[:B]
    mesh = Mesh(np.asarray(devices), ("core",))
    n_in = len(in_names) + len(out_names)
    donate_nums = tuple(range(len(in_names), n_in)) if donate else ()
    sharded = jax.jit(
        shard_map(
            _body,
            mesh=mesh,
            in_specs=(PartitionSpec("core"),) * n_in,
            out_specs=(PartitionSpec("core"),) * len(out_names),
            check_rep=False,
        ),
        donate_argnums=donate_nums,
        keep_unused=True,
    )

    state = {}

    def runner(fmap, fg_mask, bg_mask, block=True):
        key = (fmap.ctypes.data, fg_mask.ctypes.data, bg_mask.ctypes.data)
        if state.get("key") != key:
            state["key"] = key
            state["in"] = [
                jax.device_put(fmap.reshape(B * C, N)),
                jax.device_put(fg_mask.reshape(B * I_FG, N)),
                jax.device_put(bg_mask.reshape(B * Q_BG, N)),
            ]
            if not donate:
                state["zeros"] = [
                    jax.device_put(np.zeros((B * I_FG, C), np.float32)),
                    jax.device_put(np.zeros((B * Q_BG, C), np.float32)),
                ]
        if donate:
            zeros = [
                np.zeros((B * I_FG, C), np.float32),
                np.zeros((B * Q_BG, C), np.float32),
            ]
        else:
            zeros = state["zeros"]
        outs = sharded(*state["in"], *zeros)
        if block:
            jax.block_until_ready(outs)
        return outs

    _runner_cache[key] = runner
    return runner


def _make_chain_runner(n_chain):
    """One jit that executes the bass NEFF n_chain times, chaining each call's
    outputs into the next call's output-seed operands (data dependency defeats
    CSE, so XLA runs them sequentially). One dispatch total — the per-call
    tunnel overhead cancels when comparing different n_chain."""
    if ("chain", n_chain) in _runner_cache:
        return _runner_cache[("chain", n_chain)]

    import jax
    from jax.sharding import Mesh, PartitionSpec
    from jax.experimental.shard_map import shard_map
    from concourse import bass2jax

    bass2jax.install_neuronx_cc_hook()
    nc = _get_nc(1)

    in_names = ["fmap", "fg", "bg"]
    out_names = ["fg_out", "bg_out"]
    out_avals = [
        jax.core.ShapedArray((I_FG, C), np.float32),
        jax.core.ShapedArray((Q_BG, C), np.float32),
    ]
    all_in_names = in_names + out_names
    if nc.partition_id_tensor is not None:
        all_in_names.append(nc.partition_id_tensor.name)

    def _body(*args):
        fmap_a, fg_a, bg_a, z0, z1 = args
        for _ in range(n_chain):
            operands = [fmap_a, fg_a, bg_a, z0, z1]
            if nc.partition_id_tensor is not None:
                operands.append(bass2jax.partition_id_tensor())
            z0, z1 = bass2jax._bass_exec_p.bind(
                *operands,
                out_avals=tuple(out_avals),
                in_names=tuple(all_in_names),
                out_names=tuple(out_names),
                lowering_input_output_aliases=(),
                sim_require_finite=True,
                sim_require_nnan=True,
                nc=nc,
            )
        return z0, z1

    devices = jax.devices()[:B]
    mesh = Mesh(np.asarray(devices), ("core",))
    sharded = jax.jit(
        shard_map(
            _body,
            mesh=mesh,
            in_specs=(PartitionSpec("core"),) * 5,
            out_specs=(PartitionSpec("core"),) * 2,
            check_rep=False,
        ),
        keep_unused=True,
    )

    state = {}

    def runner(fmap, fg_mask, bg_mask, block=True):
        key = (fmap.ctypes.data,)
        if state.get("key") != key:
            import jax as _jax
            state["key"] = key
            state["in"] = [
                _jax.device_put(fmap.reshape(B * C, N)),
                _jax.device_put(fg_mask.reshape(B * I_FG, N)),
                _jax.device_put(bg_mask.reshape(B * Q_BG, N)),
                _jax.device_put(np.zeros((B * I_FG, C), np.float32)),
                _jax.device_put(np.zeros((B * Q_BG, C), np.float32)),
            ]
        import jax as _jax
        outs = sharded(*state["in"])
        if block:
            _jax.block_until_ready(outs)
        return outs

    _runner_cache[("chain", n_chain)] = runner
    return runner


def run(fmap, fg_mask, bg_mask, **spmd_kwargs):
    """Run on 8 NeuronCores via run_bass_kernel_spmd; returns
    (fg_init, bg_init, BassKernelResults)."""
    from concourse.bass_utils import run_bass_kernel_spmd

    fmap = np.ascontiguousarray(np.asarray(fmap, dtype=np.float32))
    fg_mask = np.ascontiguousarray(np.asarray(fg_mask, dtype=np.float32))
    bg_mask = np.ascontiguousarray(np.asarray(bg_mask, dtype=np.float32))
    assert fmap.shape == (B, C, H, W)
    assert fg_mask.shape == (B, I_FG, H, W)
    assert bg_mask.shape == (B, Q_BG, H, W)

    in_maps = [
        {
            "fmap": fmap[b].reshape(C, N),
            "fg": fg_mask[b].reshape(I_FG, N),
            "bg": bg_mask[b].reshape(Q_BG, N),
        }
        for b in range(B)
    ]
    nc = _get_nc(1)
    res = run_bass_kernel_spmd(nc, in_maps, core_ids=list(range(B)), **spmd_kwargs)
    fg_init = np.stack([r["fg_out"] for r in res.results]).astype(np.float32)
    bg_init = np.stack([r["bg_out"] for r in res.results]).astype(np.float32)
    return fg_init, bg_init, res


def kernel(fmap, fg_mask, bg_mask):
    fg_init, bg_init, _ = run(fmap, fg_mask, bg_mask)
    return fg_init, bg_init
